# revision 118
# baseline (speedup 1.0000x reference)
"""Causal multi-head self-attention on 8 Trainium2 NeuronCores.

Problem: x[4,2048,1024], Wq/Wk/Wv/Wo[1024,1024], H=16 heads, dk=64.
  q = x@Wq.T, k = x@Wk.T, v = x@Wv.T  (per-head causal softmax(q k^T/8) v) @ Wo.T

Sharding: core c handles batch b=c//2 and head-half hh=c%2 (8 heads).
Each core returns a partial output (its 512 attn columns through the
matching 512 rows of Wo.T); the host sums core pairs.

Kernel layouts (all contractions on the partition axis):
  Q/K/V projections run as fp8e4 DoubleRow matmuls (two 128-row
  contraction planes per pass, 0.5 cyc/col).  x and the weights are
  pre-scaled and split into hi+lo fp8 pairs on the host.  V accumulates
  xh*Wh + xh*Wl + xl*Wh (~15-bit effective mantissa, 6 cyc/col vs 8 for
  f32r); Q/K drop the W-lo correction (4 cyc/col) since their error
  only perturbs the softmax.  Q/K PSUM results are rescaled into single
  fp8 q8/k8 tiles laid out split-plane ([32 partitions, 2 dk-planes,
  cols] per head; W rows are pre-permuted on the host so the projection
  emits this layout directly, two 64-partition tiles keeping head
  slices at legal SBUF base partitions), which lets the score matmuls
  also run DoubleRow at 0.5 cyc/col with the dk=64 contraction packed
  as 2x32.  exp absorbs all scales.  v stays f32r (true scale restored
  during the PSUM copy), as do the PV matmuls, the O-projection and the
  normalization: softmax probabilities cannot be cheaply error-split,
  and single-fp8 there costs more accuracy than the 2e-2 budget allows.
  Outputs are written bf16 and upcast host-side; the final chunk's
  O-projection is split into two head-halves (out + out2, host-summed)
  so half of it overlaps the last attention phase instead of the tail.

  v   [s-block, head, 65]       (col 64 = ones -> softmax denominator)
  scoresT [k, q] blocks; exp on ACT (scale folds 1/8 and the fp8
  scaling, no max-subtraction -- scores are O(1) here); causal = block
  skip at 512-col granularity, column trim + affine_select zero-fill on
  diagonal blocks; attnT accumulated in PSUM with the ones column
  giving the denominator; normalization via DVE reciprocal + a rank-1
  PE broadcast matmul; O-projection from attnT layout.

The per-chunk pipeline interleaves the next chunk's projections and the
previous chunk's O-projection into the ACT-bound attention phase as PE
"fillers"; exp covers two k-blocks per instruction; PV trails two steps
behind the score matmuls across head boundaries.  Startup warms the ACT
exp table and the PE clock gate under the input DMAs.
"""

import numpy as np

import concourse.bass as bass
import concourse.mybir as mybir
import concourse.tile as tile
from concourse.bass_utils import run_bass_kernel_spmd
from concourse.vector_clock import ScopedClock, VectorClock

B, S, D, H, DK = 4, 2048, 1024, 16, 64
HPC = H // 2          # heads per core
HD = HPC * DK         # 512 head-dim columns per core
CH = 512              # q-chunk width
NCH = S // CH         # 4
NKB = S // 128        # 16 k-blocks
F32 = mybir.dt.float32
F32R = mybir.dt.float32r
F8 = mybir.dt.float8e4
BF16 = mybir.dt.bfloat16
DR = mybir.MatmulPerfMode.DoubleRow
EXP = mybir.ActivationFunctionType.Exp

# fp8 scaling: host ships x*AX and W*AW split hi+lo; PSUM q = AX*AW*q_true.
AX = 16.0
AW = 512.0
C_QK = 2.0 ** -8      # psum -> q8/k8: q8 = 32*q_true (absmax ~110)
C_V = 1.0 / (AX * AW)  # psum -> v at true scale
EXP_SCALE = 0.125 / (32.0 * 32.0)  # scores psum = 1024*(q.k)_true


def _drain_and_barrier_split(self, tick_clock, wait_clock):
    # The stock Tile tail drain attaches every outstanding sem wait to one
    # Drain instruction; this walrus build caps sync waits per instruction
    # and rejects it.  Put each wait on its own SP nop first, then drain
    # with no waits (SP has observed everything by then).
    gc = tick_clock.global_clock
    n = len(gc)
    for proc in range(n):
        t = gc[proc]
        if t == 0:
            continue
        vc = VectorClock([0] * n)
        vc.require_at_least(proc, t)
        nop = self.nc.sync.nop(nofuse=True)
        wait_clock.add_sem_waits(nop.ins, ScopedClock({None: vc}))
    self.nc.sync.drain()
    self.nc.all_engine_barrier()
    assert self.sems is not None
    popped = self.nc._tile_sem_poison_stack.pop()
    assert popped is self._sem_poison
    self.nc.clear_and_free_semaphores(list(self.sems.allocated().values()))
    self.nc.all_engine_barrier()


def _build_kernel(ctx, tc, x8h, x8l, wq8h, wk8h, wv8h, wv8l, woT, out,
                  out2):
    nc = tc.nc
    KC2 = 4  # 256-row DoubleRow contraction chunks for the projections

    wpool = ctx.enter_context(tc.tile_pool(name="weights", bufs=1))
    kvpool = ctx.enter_context(tc.tile_pool(name="kv", bufs=1))
    xpool = ctx.enter_context(tc.tile_pool(name="x", bufs=2))
    qpool = ctx.enter_context(tc.tile_pool(name="q", bufs=2))
    epool = ctx.enter_context(tc.tile_pool(name="exp", bufs=6))
    apool = ctx.enter_context(tc.tile_pool(name="attn", bufs=2))
    opool = ctx.enter_context(tc.tile_pool(name="osb", bufs=8))
    rpool = ctx.enter_context(tc.tile_pool(name="recip", bufs=3))
    # One PSUM pool, 8 banks: sc 2x[128,1024] (4) + at 3x[65,512] (3) +
    # bc 1x[64,512] (1).  Projection/O-proj groups share the "sc" slots.
    pp = ctx.enter_context(tc.tile_pool(name="pp", bufs=2, space="PSUM"))

    # --- whole-kernel-resident tiles ---
    def wtile(pfx):
        return wpool.tile([128, KC2, 2, HD], F8, tag=pfx, name=pfx)

    wqh, wkh = wtile("wqh"), wtile("wkh")
    wvh, wvl = wtile("wvh"), wtile("wvl")
    wo = wpool.tile([128, 4, D], F32R, tag="wo")
    ones = wpool.tile([1, DK], F32R, tag="ones")
    # split-plane fp8 K for the DoubleRow score matmuls.  Two 64-partition
    # tiles (A: heads h%4<2, B: h%4>=2) keep every head slice at SBUF base
    # partition 0 or 32: [32*(h%2)+d, h//4, dk-plane, s]
    k8 = (kvpool.tile([64, 2, 2, S], F8, tag="k8a", name="k8a"),
          kvpool.tile([64, 2, 2, S], F8, tag="k8b", name="k8b"))
    v = kvpool.tile([128, NKB, HPC, DK + 1], F32R, tag="v")

    # DMA queue dispatch is ~1.3us per transfer regardless of size: load
    # each hi/lo set as ONE multi-dim DMA, not one per contraction chunk.
    # The host ships every tensor already in tile layout [p, kc, i, cols].
    def dma_x(j):
        cs = slice(j * CH, (j + 1) * CH)
        xh = xpool.tile([128, KC2, 2, CH], F8, tag="xh", name="xh")
        xl = xpool.tile([128, KC2, 2, CH], F8, tag="xl", name="xl")
        nc.gpsimd.dma_start(out=xh, in_=x8h[:, :, :, cs])
        nc.gpsimd.dma_start(out=xl, in_=x8l[:, :, :, cs])
        return xh, xl

    # Startup: one big DMA per tensor, spread across the three DMA-capable
    # queues (SP, ACT, gpsimd) in need order: x+Wq first, then Wk, Wv, Wo.
    cs0 = slice(0, CH)
    xh0 = xpool.tile([128, KC2, 2, CH], F8, tag="xh", name="xh")
    xl0 = xpool.tile([128, KC2, 2, CH], F8, tag="xl", name="xl")
    nc.sync.dma_start(out=xh0, in_=x8h[:, :, :, cs0])
    nc.scalar.dma_start(out=xl0, in_=x8l[:, :, :, cs0])
    nc.gpsimd.dma_start(out=wqh, in_=wq8h)
    nc.scalar.dma_start(out=wkh, in_=wk8h)
    nc.sync.dma_start(out=wvh, in_=wv8h)
    nc.scalar.dma_start(out=wvl, in_=wv8l)
    nc.scalar.dma_start(out=wo, in_=woT.rearrange("(c p) n -> p c n", p=128))
    ones_f32 = wpool.tile([1, DK], F32, tag="ones_f32")
    nc.vector.memset(ones_f32, 1.0)
    nc.vector.tensor_copy(ones, ones_f32)
    vcol_f32 = wpool.tile([128, NKB, HPC, 1], F32, tag="vcol_f32")
    nc.vector.memset(vcol_f32, 1.0)
    nc.vector.tensor_copy(v[:, :, :, DK:DK + 1], vcol_f32)
    warm = wpool.tile([128, 128], F32R, tag="warm")
    warm_f32 = wpool.tile([128, 128], F32, tag="warm_f32")
    nc.vector.memset(warm_f32, 0.0)
    nc.vector.tensor_copy(warm, warm_f32)
    # preload the ACT exp table set under the input DMAs (~2.7us on HW)
    rcw = rpool.tile([1, DK], F32, tag="rc", name="rcw", bufs=2)
    nc.scalar.activation(out=rcw, in_=ones_f32, func=EXP, scale=1.0)
    # hold the PE clock-gate open / absorb the cold ramp while DMAs land
    wps = pp.tile([128, 2 * CH], F32, tag="sc", name="wps")
    for r in range(28):
        nc.tensor.matmul(wps[:, (r % 2) * CH:(r % 2) * CH + 128],
                         lhsT=warm, rhs=warm, start=True, stop=True)

    def qkv_fillers(j, xh, xl, kv_tag=None):
        cs = slice(j * CH, (j + 1) * CH)
        q8 = (qpool.tile([64, 2, 2, CH], F8, name=f"q8a_{j}", tag="q8a"),
              qpool.tile([64, 2, 2, CH], F8, name=f"q8b_{j}", tag="q8b"))
        fillers = []
        dense = j == 0  # attention not running yet: borrow the sc slots

        def proj(wh, dsts, mb, nmb, tag=None):
            # q/k projection m-blocks; the host W row permutation makes
            # m-block (g, i) = heads 4g..4g+3, dk-plane i.  Two passes
            # (xh*Wh + xl*Wh): q/k errors only perturb the softmax, so the
            # W-lo correction is dropped (scores stay within the error
            # budget) and wl is unused.
            def f():
                tg = tag or ("sc" if dense else "fill")
                ps = pp.tile([128, nmb * CH], F32, tag=tg,
                             bufs=None if tg == "sc" else 1, name="psf")
                for t in range(nmb):
                    ms = slice((mb + t) * 128, (mb + t + 1) * 128)
                    for wt, xt in ((wh, xh), (wh, xl)):
                        for kc in range(KC2):
                            nc.tensor.matmul(
                                ps[:, t * CH:(t + 1) * CH],
                                lhsT=wt[:, kc, :, ms], rhs=xt[:, kc, :, :],
                                start=(xt is xh and kc == 0),
                                stop=(xt is xl and kc == KC2 - 1),
                                perf_mode=DR)
                for eng, dst, psv in dsts:
                    if eng is nc.scalar:
                        eng.mul(dst, psv(ps), C_QK)
                    else:
                        eng.tensor_scalar_mul(dst, psv(ps), C_QK)
            return f

        def qk_dsts(dstp, mb, nmb):
            # dstp: (A, B) tiles [64, 2(g), 2(plane), cols].  The two
            # halves copy out on DVE and Pool in parallel so the shared
            # "fill" PSUM bank drains in one copy latency, not two.
            g = mb // 2
            if nmb == 2:
                return [
                    (nc.vector, dstp[0][:, g, :, :], lambda ps: ps.rearrange(
                        "p (t c) -> p t c", t=2)[0:64]),
                    (nc.vector, dstp[1][:, g, :, :], lambda ps: ps.rearrange(
                        "p (t c) -> p t c", t=2)[64:128]),
                ]
            i = mb % 2
            return [
                (nc.vector, dstp[0][:, g, i, :], lambda ps: ps[0:64, :]),
                (nc.vector, dstp[1][:, g, i, :], lambda ps: ps[64:128, :]),
            ]

        def vproj(sb, nsb, tag=None):
            def f():
                tg = tag or ("sc" if dense else "fill")
                ps = pp.tile([128, nsb * CH], F32, tag=tg,
                             bufs=None if tg == "sc" else 1, name="psf")
                for t in range(nsb):
                    ss = slice((sb + t) * 128, (sb + t + 1) * 128)
                    for xt, wt in ((xh, wvh), (xh, wvl), (xl, wvh)):
                        for kc in range(KC2):
                            nc.tensor.matmul(
                                ps[:, t * CH:(t + 1) * CH],
                                lhsT=xt[:, kc, :, ss], rhs=wt[:, kc, :, :],
                                start=(xt is xh and wt is wvh and kc == 0),
                                stop=(xt is xl and kc == KC2 - 1),
                                perf_mode=DR)
                sblk = j * 4 + sb
                nc.vector.tensor_scalar_mul(
                    v[:, sblk:sblk + nsb, :, 0:DK],
                    ps.rearrange("p (t h d) -> p t h d", t=nsb, h=HPC), C_V)
            return f

        nm = 2 if dense else 1
        qkcost = nm * 853   # 8 DR matmuls x 256 cyc per m-block, ~ns
        vcost = nm * 1280   # 12 DR matmuls
        for mb in range(0, 4, nm):
            fillers.append((qkcost, proj(wqh, qk_dsts(q8, mb, nm), mb, nm)))
        kv = []  # (deadline (h, g) in chunk j's own attention loop, cost, fn)
        k8cs = (k8[0][:, :, :, cs], k8[1][:, :, :, cs])
        for mb in range(0, 4, nm):
            # k8 m-block (g, i) is first read by head 4g at its step g=2j
            kv.append(((4 * (mb // 2), 2 * j - 1), qkcost,
                       proj(wkh, qk_dsts(k8cs, mb, nm), mb, nm, tag=kv_tag)))
        for sb in range(0, 4, nm):
            # v s-block 4j+sb is first read by the pv pair emitted at
            # step g = 2j + sb//2 + 2 of head 0
            kv.append(((0, 2 * j + sb // 2 + 1), vcost,
                       vproj(sb, nm, tag=kv_tag)))
        return q8, fillers, kv

    def o_fillers(j, ach):
        def oblk(sb, n):
            def f():
                sblk = j * 4 + sb
                osb = opool.tile([128, CH], BF16, name="osb", tag="osb")
                ps = pp.tile([128, CH], F32, tag="fill", bufs=1, name="psf")
                for hp in range(4):
                    nc.tensor.matmul(
                        ps, lhsT=ach[:, hp, sb * 128:(sb + 1) * 128],
                        rhs=wo[:, hp, n * CH:(n + 1) * CH],
                        start=(hp == 0), stop=(hp == 3))
                nc.vector.tensor_copy(osb, ps)
                nc.sync.dma_start(
                    out=out[sblk * 128:(sblk + 1) * 128,
                            n * CH:(n + 1) * CH], in_=osb)
            return f
        return [(853, oblk(sb, n)) for sb in range(4) for n in range(2)]

    pending_norm = []

    def _norm_one(at_ps, dst):
        # Normalize a finished head: recip of the denominator row, rank-1
        # PE broadcast across the 64 dk partitions, multiply into attnT.
        # (GPSIMD cannot touch PSUM on real hardware, so every step that
        # reads at_ps/bc must stay on DVE.)
        rc = rpool.tile([1, CH], F32R, name="rc", tag="rc", bufs=2)
        with nc.allow_low_precision(reason="f32r feed for PE broadcast"):
            nc.vector.reciprocal(out=rc, in_=at_ps[DK:DK + 1, :])
        bc = pp.tile([DK, CH], F32, tag="at", bufs=3, name="bc")
        nc.tensor.matmul(bc, lhsT=ones, rhs=rc, start=True, stop=True)
        bcs = rpool.tile([DK, CH], F32, tag="bcs", bufs=2, name="bcs")
        nc.vector.tensor_copy(bcs, bc)
        nc.vector.tensor_mul(dst, at_ps[0:DK, :], bcs)

    from collections import deque
    # entries: (kind "now"|"next", deadline or None, est PE ns, fn).
    # "next"-tagged fillers are carried into their own chunk's attention
    # loop (deadline-paced there) instead of running in the current one.
    fillers = deque()
    carry_kv = deque()  # (deadline, cost, fn) deferred into current chunk
    carry_next = deque()
    # Chunk 0 start: only Q and K of the first head-group are needed for
    # the first score matmul; defer the rest into chunk 0's attention loop
    # (deadline-paced) so exp starts ~10us earlier and overlaps them.
    q8, f0, kv0 = qkv_fillers(0, xh0, xl0)
    f0[0][1]()      # Q heads 0-3
    kv0[0][2]()     # K heads 0-3
    carry_kv.extend([
        (kv0[2][0], kv0[2][1], kv0[2][2]),    # V s-blocks 0-1 @ (0,1)
        (kv0[3][0], kv0[3][1], kv0[3][2]),    # V s-blocks 2-3 @ (0,2)
        ((4, -1), f0[1][0], f0[1][1]),        # Q heads 4-7
        ((4, -1), kv0[1][1], kv0[1][2]),      # K heads 4-7
    ])

    prev = None  # (j, ach) of the chunk awaiting its O-projection
    for j in range(NCH):
        # stage next chunk's x DMAs + projection fillers, and the previous
        # chunk's O-projection, to fill PE gaps in this ACT-bound phase.
        # The last chunk has a structural PE deficit (no next chunk to
        # project, the most exp work), so its K/V projections are deferred
        # into it rather than run ahead.
        if prev is not None:
            fillers.extend(("now", None, c, f) for c, f in o_fillers(*prev))
        if j + 1 < NCH:
            xh_n, xl_n = dma_x(j + 1)
            defer = j + 1 >= NCH - 2
            q8_n, fs, kv_n = qkv_fillers(j + 1, xh_n, xl_n)
            fillers.extend(("now", None, c, f) for c, f in fs)
            fillers.extend(("next" if defer else "now", dl, c, f)
                           for dl, c, f in kv_n)
        else:
            q8_n = None

        ach = apool.tile([128, 4, CH], F32R, name=f"ach{j}", tag="ach")
        nkb = 4 * (j + 1)
        steps = HPC * (nkb // 2)
        # Final chunk: its O-projection splits into two head-pair halves
        # summed by the host.  The heads 0-3 half only needs the first four
        # norms, so it runs inside this chunk's attention loop (deadline
        # paced over heads 5-7) instead of serializing into the tail.
        late_o = deque()
        if j == NCH - 1:
            def half_oblk(sb, n, hp0, dst):
                def f():
                    sblk = j * 4 + sb
                    osb = opool.tile([128, CH], BF16, name="osb", tag="osb")
                    ps = pp.tile([128, CH], F32, tag="fill", bufs=1,
                                 name="psf")
                    for hp in (hp0, hp0 + 1):
                        nc.tensor.matmul(
                            ps, lhsT=ach[:, hp, sb * 128:(sb + 1) * 128],
                            rhs=wo[:, hp, n * CH:(n + 1) * CH],
                            start=(hp == hp0), stop=(hp == hp0 + 1))
                    nc.vector.tensor_copy(osb, ps)
                    nc.sync.dma_start(
                        out=dst[sblk * 128:(sblk + 1) * 128,
                                n * CH:(n + 1) * CH], in_=osb)
                return f
            odl = [(5, 1), (5, 5), (6, 1), (6, 4), (6, 7), (7, 1),
                   (7, 3), (7, 5)]
            late_o.extend(
                (odl[sb * 2 + n], half_oblk(sb, n, 0, out))
                for sb in range(4) for n in range(2))
        supply = (sum(c for k, _, c, _ in fillers if k == "now")
                  + sum(c for _, c, _ in carry_kv))
        popped = 0
        gstep = 0

        closed = set()

        def emit_pv(ent):
            at_ps, h, pg, pe, is_last = ent
            for t in range(2):
                i = 2 * pg + t
                if i < 4 * j:
                    ql = 0
                else:
                    ql = min(128 * (i - 4 * j), CH - 256)
                nc.tensor.matmul(
                    at_ps[:, ql:], lhsT=v[:, i, h, :],
                    rhs=pe[:, t * CH + ql:(t + 1) * CH],
                    start=(i == 0), stop=(is_last and t == 1),
                    skip_group_check=True)
            if is_last:
                closed.add(at_ps.tensor.name)

        def flush_ready():
            # emit norms only for heads whose accumulation group is closed
            # (emission order defines read/write semantics under Tile)
            while pending_norm and pending_norm[0][0].tensor.name in closed:
                at_ps, dst = pending_norm.pop(0)
                _norm_one(at_ps, dst)

        pend = []
        for h in range(HPC):
            hg, hab, h2 = h // 4, (h % 4) // 2, h % 2
            prow = slice(h2 * 32, (h2 + 1) * 32)
            mb, half = h // 2, h % 2
            row = slice(half * DK, (half + 1) * DK)
            at_ps = pp.tile([DK + 1, CH], F32, tag="at", bufs=3, name="at_ps")
            for g in range(nkb // 2):
                # feed PE filler work BEFORE this step's score matmuls: the
                # scores stall on the exp pipeline (sc psum buffer), and PE
                # executes its stream in order, so work queued behind a
                # stalled matmul cannot cover the stall.
                while carry_kv and carry_kv[0][0] <= (h, g):
                    _, c, f = carry_kv.popleft()
                    f()
                    popped += c
                while late_o and late_o[0][0] <= (h, g):
                    late_o.popleft()[1]()
                gstep += 1
                target = supply * gstep * 9 // (steps * 8)
                ran_fill = False
                while popped < target and not ran_fill:
                    # chunk 0's deferred dense projections are DMA-gated:
                    # only their deadlines may pull them, or PE head-of-line
                    # blocks on the weight DMAs ahead of the first scores
                    if carry_kv and j > 0:
                        _, c, f = carry_kv.popleft()
                        f()
                        popped += c
                        ran_fill = True
                    elif fillers:
                        kind, dl, c, f = fillers.popleft()
                        if kind == "next":
                            carry_next.append((dl, c, f))
                        else:
                            f()
                            popped += c
                            ran_fill = True
                    else:
                        break
                i0 = 2 * g
                # Diagonal blocks are mostly masked: columns [0, qlo) of
                # k-block i are causally dead (q < k for the whole block),
                # so trim score/exp-mask/PV work to [qlo, CH).
                def _qlo(i):
                    if i < 4 * j:
                        return 0
                    return min(128 * (i - 4 * j), CH - 256)

                sc = pp.tile([128, 2 * CH], F32, tag="sc", name="sc")
                pair_ql = _qlo(i0)  # uniform over the pair so the single
                # exp below reads only written PSUM
                for t in range(2):
                    i = i0 + t
                    nc.tensor.matmul(
                        sc[:, t * CH + pair_ql:(t + 1) * CH],
                        lhsT=k8[hab][prow, hg, :, i * 128:(i + 1) * 128],
                        rhs=q8[hab][prow, hg, :, pair_ql:],
                        start=True, stop=True, perf_mode=DR)
                e = epool.tile([128, 2 * CH], F32R, name="e", tag="e")
                sc_v = sc.rearrange("p (t c) -> p t c", t=2)[:, :, pair_ql:]
                e_v = e.rearrange("p (t c) -> p t c", t=2)[:, :, pair_ql:]
                nc.scalar.activation(out=e_v, in_=sc_v, func=EXP,
                                     scale=EXP_SCALE)
                for t in range(2):
                    i = i0 + t
                    if i >= 4 * j:
                        # columns >= 128*(d+1) of the chunk are fully valid
                        # (q > every k in this block); columns < ql are
                        # never read by the trimmed pv.  Mask only between.
                        ql = _qlo(i)
                        hi = min(128 * (i - 4 * j + 1), CH)
                        nc.gpsimd.affine_select(
                            out=e[:, t * CH + ql:t * CH + hi],
                            in_=e[:, t * CH + ql:t * CH + hi],
                            compare_op=mybir.AluOpType.is_ge,
                            fill=0.0, base=j * CH - i * 128 + ql,
                            channel_multiplier=-1, pattern=[[1, hi - ql]])
                if len(pend) > 3:
                    emit_pv(pend.pop(0))
                flush_ready()
                pend.append((at_ps, h, g, e, g == nkb // 2 - 1))
            pending_norm.append((at_ps, ach[row, mb, :]))
        while pend:
            emit_pv(pend.pop(0))
        flush_ready()
        assert not pending_norm
        while carry_kv:
            carry_kv.popleft()[2]()
        while fillers:
            kind, dl, c, f = fillers.popleft()
            if kind == "next":
                carry_next.append((dl, c, f))
            else:
                f()
        # pops only ever inspect the queue front, so carried entries MUST be
        # in nondecreasing deadline order or a late-sorted entry (e.g. a V
        # s-block) can miss its deadline and be read before it is written
        carry_kv = deque(sorted(carry_next, key=lambda e: e[0]))
        carry_next = deque()
        prev = (j, ach)
        q8 = q8_n

    # Tail: only the heads 4-7 half of the final chunk's O-projection
    # remains (the heads 0-3 half ran inside the attention loop); it lands
    # in out2 and the host adds the two halves.
    jf, achf = prev
    for sb in range(4):
        for n in range(2):
            ps = pp.tile([128, CH], F32, tag="sc", name="ps_of")
            for hp in (2, 3):
                nc.tensor.matmul(
                    ps, lhsT=achf[:, hp, sb * 128:(sb + 1) * 128],
                    rhs=wo[:, hp, n * CH:(n + 1) * CH],
                    start=(hp == 2), stop=(hp == 3))
            osb = opool.tile([128, CH], BF16, name="osb", tag="osb")
            # ACT is idle by the tail and can read PSUM: alternating the
            # copies DVE/ACT halves the end-of-kernel copy chain
            if (sb * 2 + n) % 2 == 0:
                nc.vector.tensor_copy(osb, ps)
            else:
                nc.scalar.copy(osb, ps)
            oe = nc.sync if (sb * 2 + n) % 2 == 0 else nc.scalar
            oe.dma_start(
                out=out2[sb * 128:(sb + 1) * 128, n * CH:(n + 1) * CH],
                in_=osb)


def _split_excess_waits(nc, max_waits=1):
    # This walrus build rejects instructions carrying more than a couple of
    # sem waits ("Too many sync wait commands").  Engines execute their
    # stream in order, so excess waits can be moved onto nofuse nops placed
    # immediately before the instruction on the same engine.
    ctr = 0
    for blk in nc.m.functions[0].blocks:
        insts = blk.instructions
        out = []
        changed = False
        for inst in insts:
            si = inst.sync_info
            if si is not None and si.on_wait and len(si.on_wait) > max_waits:
                waits = list(si.on_wait)
                extra, keep = waits[:-max_waits], waits[-max_waits:]
                for gi in range(0, len(extra), max_waits):
                    ctr += 1
                    out.append(mybir.InstNoOp(
                        name=f"wsplit_{ctr}",
                        engine=inst.engine,
                        bass_nofuse=True,
                        sync_info=mybir.SyncInfo(
                            on_wait=extra[gi:gi + max_waits], on_update=[]),
                    ))
                inst.sync_info = mybir.SyncInfo(
                    on_wait=keep, on_update=si.on_update)
                changed = True
            out.append(inst)
        if changed:
            insts[:] = out


_CACHE = {}


def _get_nc(split=True):
    if "nc" in _CACHE:
        return _CACHE["nc"]
    tile.TileContext._drain_and_barrier = _drain_and_barrier_split
    nc = bass.Bass("TRN2", target_bir_lowering=False, debug=False)

    def din(name, shape, dt=F8):
        return nc.dram_tensor(name, shape, dt, kind="ExternalInput").ap()

    x8h = din("x8h", [128, 4, 2, S])
    x8l = din("x8l", [128, 4, 2, S])
    wq8h = din("wq8h", [128, 4, 2, HD])
    wk8h = din("wk8h", [128, 4, 2, HD])
    wv8h = din("wv8h", [128, 4, 2, HD])
    wv8l = din("wv8l", [128, 4, 2, HD])
    woT = din("woT", [HD, D], F32R)
    out = nc.dram_tensor("out", [S, D], BF16, kind="ExternalOutput").ap()
    out2 = nc.dram_tensor("out2", [CH, D], BF16, kind="ExternalOutput").ap()
    from contextlib import ExitStack
    with tile.TileContext(nc) as tc, ExitStack() as ctx:
        _build_kernel(ctx, tc, x8h, x8l, wq8h, wk8h, wv8h, wv8l, woT, out,
                      out2)
    if split:
        _split_excess_waits(nc)
        _CACHE["nc"] = nc
    return nc


def _dr_split(a, np_f8):
    """[R, C] f32 -> hi/lo fp8 in DoubleRow tile layout [128, R/256, 2, C]
    (partition, contraction pair, plane, cols)."""
    rows, cols = a.shape
    hi = a.astype(np_f8)
    lo = (a - hi.astype(np.float32)).astype(np_f8)

    def rear(m):
        return np.ascontiguousarray(
            m.reshape(rows // 256, 2, 128, cols).transpose(2, 0, 1, 3))
    return rear(hi), rear(lo)


def make_in_maps(x, Wq, Wk, Wv, Wo):
    import ml_dtypes
    np_f8 = ml_dtypes.float8_e4m3
    x = np.asarray(x, np.float32)
    Wq, Wk, Wv, Wo = (np.asarray(w, np.float32) for w in (Wq, Wk, Wv, Wo))
    # q/k projection W row permutation: m-block (g, i) = heads 4g..4g+3,
    # dk-plane i (32 dk each) so the PSUM emerges in split-plane layout.
    perm = np.array([(4 * g + h4) * DK + i * 32 + d
                     for g in range(2) for i in range(2)
                     for h4 in range(4) for d in range(32)])
    in_maps = []
    for c in range(8):
        b, hh = c // 2, c % 2
        cols = slice(hh * HD, (hh + 1) * HD)
        xT = np.ascontiguousarray(x[b].T) * AX
        x8h_a, x8l_a = _dr_split(xT, np_f8)
        wq8h_a, _ = _dr_split(Wq[cols, :][perm].T * AW, np_f8)
        wk8h_a, _ = _dr_split(Wk[cols, :][perm].T * AW, np_f8)
        wv8h_a, wv8l_a = _dr_split(Wv[cols, :].T * AW, np_f8)
        in_maps.append({
            "x8h": x8h_a, "x8l": x8l_a,
            "wq8h": wq8h_a, "wk8h": wk8h_a,
            "wv8h": wv8h_a, "wv8l": wv8l_a,
            "woT": np.ascontiguousarray(Wo[:, cols].T),
        })
    return in_maps


def kernel(x, Wq, Wk, Wv, Wo, _trace=False, _trace_kwargs=None):
    nc = _get_nc()
    in_maps = make_in_maps(x, Wq, Wk, Wv, Wo)
    res = run_bass_kernel_spmd(
        nc, in_maps, core_ids=list(range(8)), trace=_trace,
        **(_trace_kwargs or {}))
    outs = [np.asarray(res.results[c]["out"], np.float32) for c in range(8)]
    outs2 = [np.asarray(res.results[c]["out2"], np.float32)
             for c in range(8)]
    full = np.stack([outs[2 * b] + outs[2 * b + 1] for b in range(B)])
    full[:, S - CH:, :] += np.stack(
        [outs2[2 * b] + outs2[2 * b + 1] for b in range(B)])
    if _trace:
        _CACHE["last_results"] = res
    return full.astype(np.float32)
